# revision 2
# baseline (speedup 1.0000x reference)
"""TRN2 Bass kernel: per-class (segment) sums of pixel features.

Computes, for C=19 classes over N=524288 pixels with A=512 channels:
  mean[c]       = sum_{i: lab_i==c, valid} feat_i / max(count_c, 1)
  sum_weight[c] = count_c broadcast over A
  class_dist[c] = count_c

Strategy (data-parallel over pixels, 8 NeuronCores):
  Each core processes a contiguous shard of 65536 pixels.  Per 128-pixel
  tile a one-hot [128, C] matrix is built on the vector engine
  (iota == label, per-partition scalar compare); the tensor engine then
  accumulates onehot.T @ features [C, A] into a single PSUM bank across
  all 512 tiles of the shard.  Counts come from 19 is_equal+accum_out
  passes over the on-chip label tile.  Per-core partial sums/counts are
  summed on the host (trivial: 8 x 19 x 513 values) and divided.
"""

import functools

import numpy as np

import concourse.bacc as bacc
import concourse.mybir as mybir
from concourse.bass_utils import run_bass_kernel_spmd
from concourse.tile import TileContext

C = 19  # classes
A = 512  # feature channels
NCORES = 8
N = 524288  # total pixels
PER = N // NCORES  # pixels per core
T = PER // 128  # 128-pixel tiles per core (512)
G = 16  # tiles per DMA chunk (16 * 256KB = 4 MiB)
NCHUNK = T // G


@functools.lru_cache(maxsize=1)
def _build():
    nc = bacc.Bacc("TRN2", target_bir_lowering=False)
    feat = nc.dram_tensor("feat", [T, 128, A], mybir.dt.float32, kind="ExternalInput")
    # meta: cols [0, C) = iota row 0..18 (same every partition),
    #       cols [C, C+T) = labels transposed so col t = tile t's labels
    meta = nc.dram_tensor("meta", [128, C + T], mybir.dt.float32, kind="ExternalInput")
    sums = nc.dram_tensor("sums", [C, A], mybir.dt.float32, kind="ExternalOutput")
    cnt = nc.dram_tensor("cnt", [128, C], mybir.dt.float32, kind="ExternalOutput")

    with TileContext(nc) as tc:
        with (
            tc.tile_pool(name="sbuf", bufs=1) as pool,
            tc.tile_pool(name="psum", bufs=1, space="PSUM") as pp,
        ):
            meta_t = pool.tile([128, C + T], mybir.dt.float32, tag="meta", bufs=1)
            nc.sync.dma_start(out=meta_t[:], in_=meta[:])

            ps = pp.tile([C, A], mybir.dt.float32, tag="ps", bufs=1)

            for g in range(NCHUNK):
                ft = pool.tile([128, G * A], mybir.dt.float32, tag="ft", bufs=3)
                nc.sync.dma_start(
                    out=ft[:].rearrange("p (n m) -> p n m", n=G),
                    in_=feat[g * G : (g + 1) * G].rearrange("n p m -> p n m"),
                )
                for j in range(G):
                    t = g * G + j
                    oh = pool.tile([128, C], mybir.dt.float32, tag="oh", bufs=4)
                    nc.vector.tensor_scalar(
                        out=oh[:],
                        in0=meta_t[:, :C],
                        scalar1=meta_t[:, C + t : C + t + 1],
                        scalar2=None,
                        op0=mybir.AluOpType.is_equal,
                    )
                    nc.tensor.matmul(
                        ps[:],
                        lhsT=oh[:],
                        rhs=ft[:, j * A : (j + 1) * A],
                        start=(t == 0),
                        stop=(t == T - 1),
                    )

            # counts: cnt_t[p, c] = #{t : labels_t[p, t] == c}
            cnt_t = pool.tile([128, C], mybir.dt.float32, tag="cnt", bufs=1)
            scratch = pool.tile([128, T], mybir.dt.float32, tag="scr", bufs=1)
            for c in range(C):
                nc.vector.tensor_scalar(
                    out=scratch[:],
                    in0=meta_t[:, C:],
                    scalar1=float(c),
                    scalar2=None,
                    op0=mybir.AluOpType.is_equal,
                    op1=mybir.AluOpType.add,
                    accum_out=cnt_t[:, c : c + 1],
                )

            ssum = pool.tile([C, A], mybir.dt.float32, tag="ss", bufs=1)
            nc.vector.tensor_copy(out=ssum[:], in_=ps[:])
            nc.sync.dma_start(out=sums[:], in_=ssum[:])
            nc.sync.dma_start(out=cnt[:], in_=cnt_t[:])

    nc.compile()
    return nc


def _prep_inmaps(features, labels, ignore_label):
    features = np.ascontiguousarray(np.asarray(features, dtype=np.float32))
    labels = np.asarray(labels)
    ig = int(np.asarray(ignore_label))
    labf = labels.astype(np.float32)
    labf[np.asarray(labels) == ig] = -1.0  # ignored pixels match no class
    iota = np.broadcast_to(np.arange(C, dtype=np.float32), (128, C))
    in_maps = []
    for core in range(NCORES):
        fshard = features[core * PER : (core + 1) * PER].reshape(T, 128, A)
        lshard = labf[core * PER : (core + 1) * PER].reshape(T, 128).T  # [128, T]
        m = np.ascontiguousarray(
            np.concatenate([iota, lshard], axis=1), dtype=np.float32
        )
        in_maps.append({"feat": fshard, "meta": m})
    return in_maps


def _combine(results):
    sums = np.zeros((C, A), dtype=np.float64)
    counts = np.zeros((C,), dtype=np.float64)
    for r in results:
        sums += r["sums"].astype(np.float64)
        counts += r["cnt"].sum(axis=0, dtype=np.float64)
    amount = np.where(counts == 0, 1.0, counts)
    mean = (sums / amount[:, None]).astype(np.float32)
    counts32 = counts.astype(np.float32)
    sum_weight = np.broadcast_to(counts32[:, None], (C, A)).copy()
    class_dist = counts32
    return mean, sum_weight, class_dist


def _run(features, labels, ignore_label, trace=False, trace_cores=None):
    nc = _build()
    in_maps = _prep_inmaps(features, labels, ignore_label)
    res = run_bass_kernel_spmd(
        nc, in_maps, list(range(NCORES)), trace=trace, trace_cores=trace_cores
    )
    return _combine(res.results), res


def kernel(features, labels, ignore_label):
    out, _ = _run(features, labels, ignore_label)
    return out


# revision 3
# speedup vs baseline: 1.1767x; 1.1767x over previous
"""TRN2 Bass kernel: per-class (segment) sums of pixel features.

Computes, for C=19 classes over N=524288 pixels with A=512 channels:
  mean[c]       = sum_{i: lab_i==c, valid} feat_i / max(count_c, 1)
  sum_weight[c] = count_c broadcast over A
  class_dist[c] = count_c

Strategy (data-parallel over pixels, 8 NeuronCores):
  Each core processes a contiguous shard of 65536 pixels.  Features are
  shipped as an exact bf16 hi/lo pair (x == hi + lo to ~2^-18 relative),
  so the per-128-pixel-tile segment sum runs as two full-rate bf16
  matmuls (onehot.T @ hi, onehot.T @ lo) accumulating into one PSUM
  bank, instead of one quarter-rate fp32 matmul.  The one-hot [128, C]
  is built on the vector engine (iota == label, per-partition scalar
  compare).  Counts come from 19 is_equal+accum_out passes over the
  on-chip label tile.  Per-core partial sums/counts are summed on the
  host (trivial: 8 x 19 x 513 values) and divided.

  Within each 2048-pixel DMA chunk, partition p takes pixels
  chunk*2048 + p*16 .. +15, so every partition reads one contiguous
  16 KiB span per chunk (line-rate DMA).  The labels are permuted the
  same way on the host.
"""

import functools

import ml_dtypes
import numpy as np

import concourse.bacc as bacc
import concourse.mybir as mybir
from concourse.bass_utils import run_bass_kernel_spmd
from concourse.tile import TileContext

BF16 = ml_dtypes.bfloat16

C = 19  # classes
A = 512  # feature channels
NCORES = 8
N = 524288  # total pixels
PER = N // NCORES  # pixels per core
T = PER // 128  # 128-pixel tiles per core (512)
G = 16  # tiles per DMA chunk (2 MiB bf16 per half)
NCHUNK = T // G


@functools.lru_cache(maxsize=1)
def _build():
    nc = bacc.Bacc("TRN2", target_bir_lowering=False)
    fhi = nc.dram_tensor("fhi", [PER, A], mybir.dt.bfloat16, kind="ExternalInput")
    flo = nc.dram_tensor("flo", [PER, A], mybir.dt.bfloat16, kind="ExternalInput")
    # meta: cols [0, C) = iota 0..18, cols [C, C+T) = labels with col t
    # holding the (permuted) labels of matmul-tile t
    meta = nc.dram_tensor("meta", [128, C + T], mybir.dt.float32, kind="ExternalInput")
    sums = nc.dram_tensor("sums", [C, A], mybir.dt.float32, kind="ExternalOutput")
    cnt = nc.dram_tensor("cnt", [128, C], mybir.dt.float32, kind="ExternalOutput")

    with TileContext(nc) as tc:
        with (
            tc.tile_pool(name="sbuf", bufs=1) as pool,
            tc.tile_pool(name="psum", bufs=1, space="PSUM") as pp,
        ):
            meta_t = pool.tile([128, C + T], mybir.dt.float32, tag="meta", bufs=1)
            nc.sync.dma_start(out=meta_t[:], in_=meta[:])

            ps = pp.tile([C, A], mybir.dt.float32, tag="ps", bufs=1)

            for g in range(NCHUNK):
                ht = pool.tile([128, G * A], mybir.dt.bfloat16, tag="ht", bufs=3)
                lt = pool.tile([128, G * A], mybir.dt.bfloat16, tag="lt", bufs=3)
                sl = slice(g * G * 128, (g + 1) * G * 128)
                nc.sync.dma_start(
                    out=ht[:], in_=fhi[sl].rearrange("(p g) m -> p (g m)", p=128)
                )
                nc.sync.dma_start(
                    out=lt[:], in_=flo[sl].rearrange("(p g) m -> p (g m)", p=128)
                )
                for j in range(G):
                    t = g * G + j
                    oh = pool.tile([128, C], mybir.dt.bfloat16, tag="oh", bufs=4)
                    nc.vector.tensor_scalar(
                        out=oh[:],
                        in0=meta_t[:, :C],
                        scalar1=meta_t[:, C + t : C + t + 1],
                        scalar2=None,
                        op0=mybir.AluOpType.is_equal,
                    )
                    nc.tensor.matmul(
                        ps[:],
                        lhsT=oh[:],
                        rhs=ht[:, j * A : (j + 1) * A],
                        start=(t == 0),
                        stop=False,
                    )
                    nc.tensor.matmul(
                        ps[:],
                        lhsT=oh[:],
                        rhs=lt[:, j * A : (j + 1) * A],
                        start=False,
                        stop=(t == T - 1),
                    )

            # counts: cnt_t[p, c] = #{t : labels_t[p, t] == c}
            cnt_t = pool.tile([128, C], mybir.dt.float32, tag="cnt", bufs=1)
            scratch = pool.tile([128, T], mybir.dt.float32, tag="scr", bufs=1)
            for c in range(C):
                nc.vector.tensor_scalar(
                    out=scratch[:],
                    in0=meta_t[:, C:],
                    scalar1=float(c),
                    scalar2=None,
                    op0=mybir.AluOpType.is_equal,
                    op1=mybir.AluOpType.add,
                    accum_out=cnt_t[:, c : c + 1],
                )

            ssum = pool.tile([C, A], mybir.dt.float32, tag="ss", bufs=1)
            nc.vector.tensor_copy(out=ssum[:], in_=ps[:])
            nc.sync.dma_start(out=sums[:], in_=ssum[:])
            nc.sync.dma_start(out=cnt[:], in_=cnt_t[:])

    nc.compile()
    return nc


def _prep_inmaps(features, labels, ignore_label):
    f = np.asarray(features, dtype=np.float32)
    hi = f.astype(BF16)
    lo = (f - hi.astype(np.float32)).astype(BF16)
    labels = np.asarray(labels)
    ig = int(np.asarray(ignore_label))
    labf = labels.astype(np.float32)
    labf[labels == ig] = -1.0  # ignored pixels match no class
    iota = np.broadcast_to(np.arange(C, dtype=np.float32), (128, C))
    in_maps = []
    for core in range(NCORES):
        sl = slice(core * PER, (core + 1) * PER)
        # labels arranged so col t = labels of matmul-tile t under the
        # contiguous-DMA pixel permutation (pixel = chunk*2048 + p*G + j)
        lshard = (
            labf[sl].reshape(NCHUNK, 128, G).transpose(1, 0, 2).reshape(128, T)
        )
        m = np.ascontiguousarray(
            np.concatenate([iota, lshard], axis=1), dtype=np.float32
        )
        in_maps.append({"fhi": hi[sl], "flo": lo[sl], "meta": m})
    return in_maps


def _combine(results):
    sums = np.zeros((C, A), dtype=np.float64)
    counts = np.zeros((C,), dtype=np.float64)
    for r in results:
        sums += r["sums"].astype(np.float64)
        counts += r["cnt"].sum(axis=0, dtype=np.float64)
    amount = np.where(counts == 0, 1.0, counts)
    mean = (sums / amount[:, None]).astype(np.float32)
    counts32 = counts.astype(np.float32)
    sum_weight = np.broadcast_to(counts32[:, None], (C, A)).copy()
    class_dist = counts32
    return mean, sum_weight, class_dist


def _run(features, labels, ignore_label, trace=False, trace_cores=None):
    nc = _build()
    in_maps = _prep_inmaps(features, labels, ignore_label)
    res = run_bass_kernel_spmd(
        nc, in_maps, list(range(NCORES)), trace=trace, trace_cores=trace_cores
    )
    return _combine(res.results), res


def kernel(features, labels, ignore_label):
    out, _ = _run(features, labels, ignore_label)
    return out


# revision 5
# speedup vs baseline: 1.1825x; 1.0049x over previous
"""TRN2 Bass kernel: per-class (segment) sums of pixel features.

Computes, for C=19 classes over N=524288 pixels with A=512 channels:
  mean[c]       = sum_{i: lab_i==c, valid} feat_i / max(count_c, 1)
  sum_weight[c] = count_c broadcast over A
  class_dist[c] = count_c

Strategy (data-parallel over pixels, 8 NeuronCores):
  Each core processes a contiguous shard of 65536 pixels.  Features are
  shipped as an exact bf16 hi/lo pair (x == hi + lo to ~2^-18 relative),
  so the per-128-pixel-tile segment sum runs as two full-rate bf16
  matmuls (onehot.T @ hi, onehot.T @ lo) accumulating into one PSUM
  bank, instead of one quarter-rate fp32 matmul.  The one-hot [128, C]
  is built on the vector engine (iota == label, per-partition scalar
  compare).  Counts come from 19 is_equal+accum_out passes over the
  on-chip label tile.  Per-core partial sums/counts are summed on the
  host (trivial: 8 x 19 x 513 values) and divided.

  Within each 2048-pixel DMA chunk, partition p takes pixels
  chunk*2048 + p*16 .. +15, so every partition reads one contiguous
  16 KiB span per chunk (line-rate DMA).  The labels are permuted the
  same way on the host.
"""

import functools

import ml_dtypes
import numpy as np

import concourse.bacc as bacc
import concourse.mybir as mybir
from concourse.bass_utils import run_bass_kernel_spmd
from concourse.tile import TileContext

BF16 = ml_dtypes.bfloat16

C = 19  # classes
A = 512  # feature channels
NCORES = 8
N = 524288  # total pixels
PER = N // NCORES  # pixels per core
T = PER // 128  # 128-pixel tiles per core (512)
G = 16  # tiles per DMA chunk (2 MiB bf16 per half)
NCHUNK = T // G


@functools.lru_cache(maxsize=1)
def _build():
    nc = bacc.Bacc("TRN2", target_bir_lowering=False)
    fhi = nc.dram_tensor("fhi", [PER, A], mybir.dt.bfloat16, kind="ExternalInput")
    flo = nc.dram_tensor("flo", [PER, A], mybir.dt.bfloat16, kind="ExternalInput")
    # meta: cols [0, C) = iota 0..18, cols [C, C+T) = labels with col t
    # holding the (permuted) labels of matmul-tile t
    meta = nc.dram_tensor("meta", [128, C + T], mybir.dt.float32, kind="ExternalInput")
    sums = nc.dram_tensor("sums", [C, A], mybir.dt.float32, kind="ExternalOutput")
    cnt = nc.dram_tensor("cnt", [128, C], mybir.dt.float32, kind="ExternalOutput")

    with TileContext(nc) as tc:
        with (
            tc.tile_pool(name="sbuf", bufs=1) as pool,
            tc.tile_pool(name="psum", bufs=1, space="PSUM") as pp,
        ):
            meta_t = pool.tile([128, C + T], mybir.dt.float32, tag="meta", bufs=1)
            nc.sync.dma_start(out=meta_t[:], in_=meta[:])

            ps = pp.tile([C, A], mybir.dt.float32, tag="ps", bufs=1)

            # counts first: they only need meta_t, so the vector engine can
            # absorb them while the first feature chunks stream in, instead
            # of adding ~13us of tail after the last chunk.
            # cnt_t[p, c] = #{t : labels_t[p, t] == c}
            cnt_t = pool.tile([128, C], mybir.dt.float32, tag="cnt", bufs=1)
            scratch = pool.tile([128, T], mybir.dt.float32, tag="scr", bufs=1)
            for c in range(C):
                nc.vector.tensor_scalar(
                    out=scratch[:],
                    in0=meta_t[:, C:],
                    scalar1=float(c),
                    scalar2=None,
                    op0=mybir.AluOpType.is_equal,
                    op1=mybir.AluOpType.add,
                    accum_out=cnt_t[:, c : c + 1],
                )
            nc.sync.dma_start(out=cnt[:], in_=cnt_t[:])

            for g in range(NCHUNK):
                ht = pool.tile([128, G * A], mybir.dt.bfloat16, tag="ht", bufs=4)
                lt = pool.tile([128, G * A], mybir.dt.bfloat16, tag="lt", bufs=4)
                sl = slice(g * G * 128, (g + 1) * G * 128)
                nc.sync.dma_start(
                    out=ht[:], in_=fhi[sl].rearrange("(p g) m -> p (g m)", p=128)
                )
                nc.sync.dma_start(
                    out=lt[:], in_=flo[sl].rearrange("(p g) m -> p (g m)", p=128)
                )
                for j in range(G):
                    t = g * G + j
                    oh = pool.tile([128, C], mybir.dt.bfloat16, tag="oh", bufs=4)
                    nc.vector.tensor_scalar(
                        out=oh[:],
                        in0=meta_t[:, :C],
                        scalar1=meta_t[:, C + t : C + t + 1],
                        scalar2=None,
                        op0=mybir.AluOpType.is_equal,
                    )
                    nc.tensor.matmul(
                        ps[:],
                        lhsT=oh[:],
                        rhs=ht[:, j * A : (j + 1) * A],
                        start=(t == 0),
                        stop=False,
                    )
                    nc.tensor.matmul(
                        ps[:],
                        lhsT=oh[:],
                        rhs=lt[:, j * A : (j + 1) * A],
                        start=False,
                        stop=(t == T - 1),
                    )

            ssum = pool.tile([C, A], mybir.dt.float32, tag="ss", bufs=1)
            nc.vector.tensor_copy(out=ssum[:], in_=ps[:])
            nc.sync.dma_start(out=sums[:], in_=ssum[:])

    nc.compile()
    return nc


def _prep_inmaps(features, labels, ignore_label):
    f = np.asarray(features, dtype=np.float32)
    hi = f.astype(BF16)
    lo = (f - hi.astype(np.float32)).astype(BF16)
    labels = np.asarray(labels)
    ig = int(np.asarray(ignore_label))
    labf = labels.astype(np.float32)
    labf[labels == ig] = -1.0  # ignored pixels match no class
    iota = np.broadcast_to(np.arange(C, dtype=np.float32), (128, C))
    in_maps = []
    for core in range(NCORES):
        sl = slice(core * PER, (core + 1) * PER)
        # labels arranged so col t = labels of matmul-tile t under the
        # contiguous-DMA pixel permutation (pixel = chunk*2048 + p*G + j)
        lshard = (
            labf[sl].reshape(NCHUNK, 128, G).transpose(1, 0, 2).reshape(128, T)
        )
        m = np.ascontiguousarray(
            np.concatenate([iota, lshard], axis=1), dtype=np.float32
        )
        in_maps.append({"fhi": hi[sl], "flo": lo[sl], "meta": m})
    return in_maps


def _combine(results):
    sums = np.zeros((C, A), dtype=np.float64)
    counts = np.zeros((C,), dtype=np.float64)
    for r in results:
        sums += r["sums"].astype(np.float64)
        counts += r["cnt"].sum(axis=0, dtype=np.float64)
    amount = np.where(counts == 0, 1.0, counts)
    mean = (sums / amount[:, None]).astype(np.float32)
    counts32 = counts.astype(np.float32)
    sum_weight = np.broadcast_to(counts32[:, None], (C, A)).copy()
    class_dist = counts32
    return mean, sum_weight, class_dist


def _run(features, labels, ignore_label, trace=False, trace_cores=None):
    nc = _build()
    in_maps = _prep_inmaps(features, labels, ignore_label)
    res = run_bass_kernel_spmd(
        nc, in_maps, list(range(NCORES)), trace=trace, trace_cores=trace_cores
    )
    return _combine(res.results), res


def kernel(features, labels, ignore_label):
    out, _ = _run(features, labels, ignore_label)
    return out
